# revision 9
# baseline (speedup 1.0000x reference)
"""Trainium2 Bass kernel for BatchSquareDiagonal.

Computes out[b] = sum_n d[b, n] * x[b, n]^2 for x, d of shape [16384, 2048]
f32, returning [16384, 1] f32. Pure data parallel across 8 NeuronCores:
core c handles batch rows [c*2048, (c+1)*2048).

Per-core pipeline (memory-bound; ~33.5 MB of input reads per core):
  - DMA row-tiles of x and d into SBUF ([128, G*2048] per tile group)
  - ScalarE (ACT): square x in SBUF
  - VectorE (DVE): tensor_tensor_reduce -> sum(x^2 * d) per partition,
    elementwise product discarded into a stride-0 broadcast dummy
  - one final DMA of the [128, 16] result block to DRAM
"""

import os
import sys

import numpy as np

for _p in ("/opt/trn_rl_repo", os.path.expanduser("~/.axon_site/_ro/trn_rl_repo")):
    if os.path.isdir(_p) and _p not in sys.path:
        sys.path.insert(0, _p)

N_CORES = 8
B, N = 16384, 2048
B_LOCAL = B // N_CORES  # 2048 rows per core
P = 128                 # SBUF partitions
G = 2                   # 128-row blocks per DMA group
N_TILES = B_LOCAL // P  # 16
N_GROUPS = N_TILES // G

_NC_CACHE = {}


def _build_nc():
    import concourse.bass as bass
    import concourse.tile as tile
    from concourse import bacc, mybir

    f32 = mybir.dt.float32
    # Bacc (not raw Bass): its compile() runs generate_event_semaphores,
    # which splits multi-sem waits (TRN2 allows 1 wait per instruction).
    nc = bacc.Bacc("TRN2", target_bir_lowering=False, debug=False)
    x = nc.declare_dram_parameter("vector", [B_LOCAL, N], f32, isOutput=False)
    d = nc.declare_dram_parameter("diag_values", [B_LOCAL, N], f32, isOutput=False)
    out = nc.declare_dram_parameter("out", [B_LOCAL, 1], f32, isOutput=True)

    # [N_TILES, P, N] views: tile j covers rows 128*j .. 128*j+127
    xv = x.ap().rearrange("(t p) n -> t p n", p=P)
    dv = d.ap().rearrange("(t p) n -> t p n", p=P)
    # out[128*j + p] == res[p, j]
    outv = out.ap().rearrange("(j p) o -> p (j o)", p=P)

    with tile.TileContext(nc) as tc:
        with (
            tc.tile_pool(name="io", bufs=6) as io_pool,
            tc.tile_pool(name="acc", bufs=1) as acc_pool,
        ):
            res = acc_pool.tile([P, N_TILES], f32)
            for j in range(N_TILES):
                xt = io_pool.tile([P, N], f32, tag="x")
                dt = io_pool.tile([P, N], f32, tag="d")
                nc.sync.dma_start(out=xt, in_=xv[j])
                nc.sync.dma_start(out=dt, in_=dv[j])
                # in-place: xt <- x^2 (ACT), xt <- xt*d (DVE), then row-sum
                nc.scalar.square(xt, xt)
                nc.vector.tensor_mul(xt, xt, dt)
                nc.vector.tensor_reduce(
                    out=res[:, j : j + 1],
                    in_=xt,
                    axis=mybir.AxisListType.X,
                    op=mybir.AluOpType.add,
                )
                # overlap output stores with compute; the last one only
                # waits on the last reduce (512 B -> rows 128j..128j+127).
                # gpsimd SWDGE queue: keeps the sync HWDGE ring free for
                # loads (a store's sem wait would stall later loads there).
                nc.gpsimd.dma_start(out=outv[:, j : j + 1], in_=res[:, j : j + 1])

    # Bacc.finalize() -> compile() (event-sem wait splitting, extended-ISA
    # codegen) + freeze. run_bass_via_pjrt serializes nc.m as-is, so this
    # must happen here.
    nc.finalize()
    return nc


def _get_nc():
    if "nc" not in _NC_CACHE:
        _NC_CACHE["nc"] = _build_nc()
    return _NC_CACHE["nc"]


def kernel(vector, diag_values):
    from concourse.bass_utils import run_bass_kernel_spmd

    vector = np.ascontiguousarray(np.asarray(vector, dtype=np.float32))
    diag_values = np.ascontiguousarray(np.asarray(diag_values, dtype=np.float32))
    assert vector.shape == (B, N) and diag_values.shape == (B, N)

    vs = vector.reshape(N_CORES, B_LOCAL, N)
    ds = diag_values.reshape(N_CORES, B_LOCAL, N)
    in_maps = [{"vector": vs[c], "diag_values": ds[c]} for c in range(N_CORES)]

    nc = _get_nc()
    res = run_bass_kernel_spmd(nc, in_maps, list(range(N_CORES)))
    return np.concatenate([res.results[c]["out"] for c in range(N_CORES)], axis=0)


# revision 10
# speedup vs baseline: 1.4452x; 1.4452x over previous
"""Trainium2 Bass kernel for BatchSquareDiagonal.

Computes out[b] = sum_n d[b, n] * x[b, n]^2 for x, d of shape [16384, 2048]
f32, returning [16384, 1] f32. Pure data parallel across 8 NeuronCores:
core c handles batch rows [c*2048, (c+1)*2048).

Per-core pipeline (memory-bound; ~33.5 MB of input reads per core):
  - DMA row-tiles of x and d into SBUF ([128, G*2048] per tile group)
  - ScalarE (ACT): square x in SBUF
  - VectorE (DVE): tensor_tensor_reduce -> sum(x^2 * d) per partition,
    elementwise product discarded into a stride-0 broadcast dummy
  - one final DMA of the [128, 16] result block to DRAM
"""

import os
import sys

import numpy as np

for _p in ("/opt/trn_rl_repo", os.path.expanduser("~/.axon_site/_ro/trn_rl_repo")):
    if os.path.isdir(_p) and _p not in sys.path:
        sys.path.insert(0, _p)

N_CORES = 8
B, N = 16384, 2048
B_LOCAL = B // N_CORES  # 2048 rows per core
P = 128                 # SBUF partitions
G = 2                   # 128-row blocks per DMA group
N_TILES = B_LOCAL // P  # 16
N_GROUPS = N_TILES // G

_NC_CACHE = {}


def _build_nc():
    import concourse.bass as bass
    import concourse.tile as tile
    from concourse import bacc, mybir

    f32 = mybir.dt.float32
    # Bacc (not raw Bass): its compile() runs generate_event_semaphores,
    # which splits multi-sem waits (TRN2 allows 1 wait per instruction).
    nc = bacc.Bacc("TRN2", target_bir_lowering=False, debug=False)
    x = nc.declare_dram_parameter("vector", [B_LOCAL, N], f32, isOutput=False)
    d = nc.declare_dram_parameter("diag_values", [B_LOCAL, N], f32, isOutput=False)
    out = nc.declare_dram_parameter("out", [B_LOCAL, 1], f32, isOutput=True)

    # [N_TILES, P, N] views: tile j covers rows 128*j .. 128*j+127
    xv = x.ap().rearrange("(t p) n -> t p n", p=P)
    dv = d.ap().rearrange("(t p) n -> t p n", p=P)
    # out[128*j + p] == res[p, j]
    outv = out.ap().rearrange("(j p) o -> p (j o)", p=P)

    with tile.TileContext(nc) as tc:
        with (
            tc.tile_pool(name="io", bufs=6) as io_pool,
            tc.tile_pool(name="acc", bufs=N_TILES) as acc_pool,
        ):
            for j in range(N_TILES):
                xt = io_pool.tile([P, N], f32, tag="x")
                dt = io_pool.tile([P, N], f32, tag="d")
                nc.sync.dma_start(out=xt, in_=xv[j])
                nc.sync.dma_start(out=dt, in_=dv[j])
                # xt's only reader is the square -> its slot frees early,
                # so the load of tile j+bufs isn't gated on tile j's full
                # compute chain (that stalls the FIFO sync ring).
                sq = io_pool.tile([P, N], f32, tag="sq")
                nc.scalar.square(sq, xt)
                nc.vector.tensor_mul(sq, sq, dt)
                # per-tile result tile: no false WAR between tiles on a
                # shared res buffer
                rj = acc_pool.tile([P, 1], f32, tag="res")
                nc.vector.tensor_reduce(
                    out=rj,
                    in_=sq,
                    axis=mybir.AxisListType.X,
                    op=mybir.AluOpType.add,
                )
                # overlap output stores with compute; the last one only
                # waits on the last reduce (512 B -> rows 128j..128j+127).
                # gpsimd SWDGE queue: keeps the sync HWDGE ring free for
                # loads (a store's sem wait would stall later loads there).
                nc.gpsimd.dma_start(out=outv[:, j : j + 1], in_=rj)

    # Bacc.finalize() -> compile() (event-sem wait splitting, extended-ISA
    # codegen) + freeze. run_bass_via_pjrt serializes nc.m as-is, so this
    # must happen here.
    nc.finalize()
    return nc


def _get_nc():
    if "nc" not in _NC_CACHE:
        _NC_CACHE["nc"] = _build_nc()
    return _NC_CACHE["nc"]


def kernel(vector, diag_values):
    from concourse.bass_utils import run_bass_kernel_spmd

    vector = np.ascontiguousarray(np.asarray(vector, dtype=np.float32))
    diag_values = np.ascontiguousarray(np.asarray(diag_values, dtype=np.float32))
    assert vector.shape == (B, N) and diag_values.shape == (B, N)

    vs = vector.reshape(N_CORES, B_LOCAL, N)
    ds = diag_values.reshape(N_CORES, B_LOCAL, N)
    in_maps = [{"vector": vs[c], "diag_values": ds[c]} for c in range(N_CORES)]

    nc = _get_nc()
    res = run_bass_kernel_spmd(nc, in_maps, list(range(N_CORES)))
    return np.concatenate([res.results[c]["out"] for c in range(N_CORES)], axis=0)


# revision 11
# speedup vs baseline: 2.3471x; 1.6240x over previous
"""Trainium2 Bass kernel for BatchSquareDiagonal.

Computes out[b] = sum_n d[b, n] * x[b, n]^2 for x, d of shape [16384, 2048]
f32, returning [16384, 1] f32. Pure data parallel across 8 NeuronCores:
core c handles batch rows [c*2048, (c+1)*2048).

Per-core pipeline (memory-bound; ~33.5 MB of input reads per core):
  - DMA row-tiles of x and d into SBUF ([128, G*2048] per tile group)
  - ScalarE (ACT): square x in SBUF
  - VectorE (DVE): tensor_tensor_reduce -> sum(x^2 * d) per partition,
    elementwise product discarded into a stride-0 broadcast dummy
  - one final DMA of the [128, 16] result block to DRAM
"""

import os
import sys

import numpy as np

for _p in ("/opt/trn_rl_repo", os.path.expanduser("~/.axon_site/_ro/trn_rl_repo")):
    if os.path.isdir(_p) and _p not in sys.path:
        sys.path.insert(0, _p)

N_CORES = 8
B, N = 16384, 2048
B_LOCAL = B // N_CORES  # 2048 rows per core
P = 128                 # SBUF partitions
G = 2                   # 128-row blocks per DMA group
N_TILES = B_LOCAL // P  # 16
N_GROUPS = N_TILES // G

_NC_CACHE = {}


def _build_nc():
    import concourse.bass as bass
    import concourse.tile as tile
    from concourse import bacc, mybir

    f32 = mybir.dt.float32
    # Bacc (not raw Bass): its compile() runs generate_event_semaphores,
    # which splits multi-sem waits (TRN2 allows 1 wait per instruction).
    nc = bacc.Bacc("TRN2", target_bir_lowering=False, debug=False)
    x = nc.declare_dram_parameter("vector", [B_LOCAL, N], f32, isOutput=False)
    d = nc.declare_dram_parameter("diag_values", [B_LOCAL, N], f32, isOutput=False)
    out = nc.declare_dram_parameter("out", [B_LOCAL, 1], f32, isOutput=True)

    # [N_TILES, P, N] views: tile j covers rows 128*j .. 128*j+127
    xv = x.ap().rearrange("(t p) n -> t p n", p=P)
    dv = d.ap().rearrange("(t p) n -> t p n", p=P)
    # out[128*j + p] == res[p, j]
    outv = out.ap().rearrange("(j p) o -> p (j o)", p=P)

    with tile.TileContext(nc) as tc:
        with (
            tc.tile_pool(name="io", bufs=3) as io_pool,
            tc.tile_pool(name="acc", bufs=1) as acc_pool,
        ):
            res = acc_pool.tile([P, N_TILES], f32)

            # Groups 0..6: [128, 2, 2048] double-row-block tiles, 2 MB DMAs
            for g in range(N_GROUPS - 1):
                xt = io_pool.tile([P, G * N], f32, tag="x", bufs=4)
                dt = io_pool.tile([P, G * N], f32, tag="d", bufs=4)
                sq = io_pool.tile([P, G * N], f32, tag="sq", bufs=3)
                xg = xv[G * g : G * g + G].transpose([1, 0, 2])
                dg = dv[G * g : G * g + G].transpose([1, 0, 2])
                nc.sync.dma_start(out=xt.rearrange("p (i n) -> p i n", i=G), in_=xg)
                nc.sync.dma_start(out=dt.rearrange("p (i n) -> p i n", i=G), in_=dg)
                nc.scalar.square(sq, xt)
                nc.vector.tensor_mul(sq, sq, dt)
                for i in range(G):
                    j = G * g + i
                    nc.vector.tensor_reduce(
                        out=res[:, j : j + 1],
                        in_=sq[:, bass.ts(i, N)],
                        axis=mybir.AxisListType.X,
                        op=mybir.AluOpType.add,
                    )

            # Tile 14: single row block (1 MB DMAs) to shorten the pipeline tail
            x14 = io_pool.tile([P, N], f32, tag="x", bufs=4)
            d14 = io_pool.tile([P, N], f32, tag="d", bufs=4)
            s14 = io_pool.tile([P, N], f32, tag="sq", bufs=3)
            nc.sync.dma_start(out=x14, in_=xv[14])
            nc.sync.dma_start(out=d14, in_=dv[14])
            nc.scalar.square(s14, x14)
            nc.vector.tensor_mul(s14, s14, d14)
            nc.vector.tensor_reduce(
                out=res[:, 14:15], in_=s14,
                axis=mybir.AxisListType.X, op=mybir.AluOpType.add,
            )

            # Tile 15 (last): load x whole, d in column halves so the
            # post-last-byte chain is only a half-width mul+reduce (~2.5us)
            H = N // 2
            x15 = io_pool.tile([P, N], f32, tag="x", bufs=4)
            d15 = io_pool.tile([P, N], f32, tag="d", bufs=4)
            s15 = io_pool.tile([P, N], f32, tag="sq", bufs=3)
            r15 = acc_pool.tile([P, 2], f32)
            nc.sync.dma_start(out=x15, in_=xv[15])
            nc.sync.dma_start(out=d15[:, :H], in_=dv[15][:, :H])
            nc.sync.dma_start(out=d15[:, H:], in_=dv[15][:, H:])
            nc.scalar.square(s15, x15)
            for h in range(2):
                cs = slice(h * H, (h + 1) * H)
                nc.vector.tensor_mul(s15[:, cs], s15[:, cs], d15[:, cs])
                nc.vector.tensor_reduce(
                    out=r15[:, h : h + 1], in_=s15[:, cs],
                    axis=mybir.AxisListType.X, op=mybir.AluOpType.add,
                )
            nc.vector.tensor_add(res[:, 15:16], r15[:, 0:1], r15[:, 1:2])

            # Store cols 0..14 early (overlaps tail compute), col 15 last
            nc.gpsimd.dma_start(out=outv[:, :15], in_=res[:, :15])
            nc.gpsimd.dma_start(out=outv[:, 15:16], in_=res[:, 15:16])

    # Bacc.finalize() -> compile() (event-sem wait splitting, extended-ISA
    # codegen) + freeze. run_bass_via_pjrt serializes nc.m as-is, so this
    # must happen here.
    nc.finalize()
    return nc


def _get_nc():
    if "nc" not in _NC_CACHE:
        _NC_CACHE["nc"] = _build_nc()
    return _NC_CACHE["nc"]


def kernel(vector, diag_values):
    from concourse.bass_utils import run_bass_kernel_spmd

    vector = np.ascontiguousarray(np.asarray(vector, dtype=np.float32))
    diag_values = np.ascontiguousarray(np.asarray(diag_values, dtype=np.float32))
    assert vector.shape == (B, N) and diag_values.shape == (B, N)

    vs = vector.reshape(N_CORES, B_LOCAL, N)
    ds = diag_values.reshape(N_CORES, B_LOCAL, N)
    in_maps = [{"vector": vs[c], "diag_values": ds[c]} for c in range(N_CORES)]

    nc = _get_nc()
    res = run_bass_kernel_spmd(nc, in_maps, list(range(N_CORES)))
    return np.concatenate([res.results[c]["out"] for c in range(N_CORES)], axis=0)
